# revision 46
# baseline (speedup 1.0000x reference)
"""GCN classifier TRN2 kernel: gather-free L2 via interpolation matmul,
fully local table build (no AllGather), binary-S gather-based L3.

Key structure (vs a naive 3-layer gather/scatter GCN):
- h1[s] = relu(q1[s] W0 + b0) depends on ONE scalar q1[s] (the L1
  aggregate, graph-only, host-precomputed). L2's edge aggregation
  sum_e w_e h1[src_e] therefore collapses to a dense matmul C2 @ T:
  T = h1 evaluated on a B=32-point grid (computed on device from
  W0,b0), C2 = host-built linear-interpolation weights folded with
  w_e = ns[src] nd[dst] (graph-only, fp8). Exact to ~1e-3.
- z2 = W1^T agg2 folds further: z = (T W1)^T C2^T, with b1 and the
  next layer's src-norm ns handled by an extra (ns, b1) row pair in
  c2t/TW1 -- L2 is 2 matmuls + relu + copy per 4 blocks.
- Every core computes the FULL 50176-row table2 locally (~60us of
  dense work) => NO AllGather. Only a 20KB AllReduce of the final
  [512, 10] logits remains (Wc and bc/8 applied before the reduce).
- table2 rows are stored in a per-16-block (partition, block)
  interleaved order so each partition writes 16 consecutive rows =
  4KB contiguous write descriptors (2x cheaper than 256B strided).
- L3 gathers table2[src] per 1024-idx SWDGE gather (4 queues, 28-deep
  msg pool); rows are p2' = ns[s] (h2[s] @ W2) so the one-hot S is
  BINARY (one DVE is_equal per 8-chunk group; nd[dst] and 1/cnt fold
  into the post-agg activation scale; (1/nd x b2) via a K=1 matmul).
- Table rows split in two groups (table2a = first 144 blocks): the
  A-stream gathers gate on table2a only and start mid-L2. Nodes are
  dealt to (core, block) bins by in-degree (snake order) to equalize
  per-block edge counts (less gather-chunk ceil padding). i16 gather
  idx limits each group window to <= 32768 rows.
- relu/copy engine-alternate between Act and DVE per z-group; S-gen
  and readout one-hots on DVE; gathers + AllReduce on GPSIMD/Pool.

Per core: prologue (T, TW1) -> L2 392 blocks -> L3 49 dst blocks
(gather, agg matmuls, h3, per-graph readout rT += h3^T Sg) ->
partial logits -> AllReduce -> out copy. Rel err ~1.3e-3 (fp16/fp8).
"""

import sys

sys.path.insert(0, "/opt/trn_rl_repo")

import numpy as np

import concourse.bass as bass
import concourse.mybir as mybir
import concourse.tile as tile
from concourse import bacc, bass_utils

P = 128
N_CORES = 8
N_NODES = 50000
N_EDGES = 800000
HID = 128
N_GRAPHS = 512
N_CLASSES = 10
B = 32              # q1 interpolation grid size (c2t row B = ns)

NPC = 6272          # nodes per core (49 blocks of 128)
BLOCKS = NPC // P   # 49
NPAD = NPC * N_CORES  # 50176
NBLK = NPAD // P    # 392 table blocks
GB = 18             # blocks per core in table group 0 (8*GB % 16 == 0, B window <= 32768)
R0 = GB * P         # 3200 rows/core in group 0
R1 = NPC - R0       # 3072 rows/core in group 1
G0E = N_CORES * R0  # 25600 rows in group 0 (stream-A window, 200 blocks)
GA = 8              # gather group size in chunks (1024 idxs = HW cap)
F32 = mybir.dt.float32
F16 = mybir.dt.float16
I16 = mybir.dt.int16
I32 = mybir.dt.int32
F8 = mybir.dt.float8e4

TDT = F16           # gather table dtype


def _prep_graph(src, dst, graph_ids):
    """Host-side preprocessing: degrees, q1, C2 interp matrix, per-core
    edge schedule for the L3 gather."""
    src = np.asarray(src).astype(np.int64)
    dst = np.asarray(dst).astype(np.int64)
    graph_ids = np.asarray(graph_ids).astype(np.int64)

    in_deg = np.bincount(dst, minlength=N_NODES).astype(np.float64)
    out_deg = np.bincount(src, minlength=N_NODES).astype(np.float64)
    ns = np.maximum(out_deg, 1.0) ** -0.5
    nd = np.maximum(in_deg, 1.0) ** -0.5
    # layer-1 aggregate: q1 = nd * segsum_dst((in_deg*ns)[src])
    c0 = in_deg * ns
    q1 = nd * np.bincount(dst, weights=c0[src], minlength=N_NODES)

    w_e = ns[src] * nd[dst]

    # ---- balanced node -> position dealing: sort by in_deg, snake
    # round-robin over the 392 (core, block) bins so every 128-dst block
    # carries ~equal edge count (minimizes gather-chunk ceil padding) ----
    deg_pad = np.zeros(NPAD)
    deg_pad[:N_NODES] = in_deg
    order = np.argsort(-deg_pad, kind="stable")
    r = np.arange(NPAD)
    rnd = r // NBLK
    binr = r % NBLK
    bins = np.where(rnd % 2 == 0, binr, NBLK - 1 - binr)
    c_of = bins // BLOCKS
    b_of = bins % BLOCKS
    posn = c_of * NPC + b_of * P + rnd
    pos = np.empty(NPAD, np.int64)
    pos[order] = posn
    inv_pos = np.empty(NPAD, np.int64)
    inv_pos[posn] = order
    pdst = pos[dst]
    psrc = pos[src]

    # ---- C2: interpolation-weight matrix on f16-rounded uniform grid ----
    qb = np.linspace(q1.min(), q1.max(), B)
    qb = qb.astype(np.float16).astype(np.float64)  # device grid == host grid
    qe = q1[src]
    ii = np.clip(np.searchsorted(qb, qe, side="right") - 1, 0, B - 2)
    alpha = np.clip((qe - qb[ii]) / (qb[ii + 1] - qb[ii]), 0.0, 1.0)
    C2 = np.zeros((NPAD, B))
    np.add.at(C2, (pdst, ii), w_e * (1.0 - alpha))
    np.add.at(C2, (pdst, ii + 1), w_e * alpha)
    # fold the src-side norm of the NEXT layer into the row of node s:
    # table2[s] = ns[s] * relu(z2[s] + b1) @ W2 = relu(ns*z2 + ns*b1) @ W2
    nsfull = np.zeros(NPAD, np.float64)
    nsfull[pos[:N_NODES]] = ns
    C2 = C2 * nsfull[:, None]

    def remap_logical(s):
        c, r = s // NPC, s % NPC
        return np.where(r < R0, c * R0 + r, G0E + c * R1 + (r - R0))

    def remap(s):
        """node -> physical table row: core-group layout composed with the
        per-16-block write interleave (partition-major within a chunk, so
        each partition's 16 rows are contiguous => 4KB write descriptors).
        Used for gather idx; c2t columns stay in LOGICAL compute order."""
        return _interleave(remap_logical(s))

    GBLKS = G0E // P          # blocks in group A (multiple of 16)
    NB_B = NBLK - GBLKS       # blocks in group B

    def _interleave(logical):
        bt, p = logical // P, logical % P
        ba = bt - GBLKS
        # group A: chunks of 16
        c0a = bt // 16
        xa = np.minimum(16, GBLKS - c0a * 16)
        phys_a = c0a * 16 * P + p * xa + (bt - c0a * 16)
        # group B: chunks of 16 from block GBLKS
        c0b = ba // 16
        xb = np.minimum(16, NB_B - c0b * 16)
        phys_b = G0E + c0b * 16 * P + p * xb + (ba - c0b * 16)
        return np.where(bt < GBLKS, phys_a, phys_b)

    # compute-order permutation: c2t column j holds node inv_perm[j]
    # (logical order: the L2 loop computes logical blocks in sequence)
    rows = remap_logical(np.arange(NPAD))
    inv_perm = np.empty(NPAD, np.int64)
    inv_perm[rows] = np.arange(NPAD)
    # c2t row 127 = ns (pairs with TW1 row 127 = b1: the K=128 z-matmul
    # then computes TW1^T C2^T + b1 x ns in one accumulation)
    import ml_dtypes
    c2t = np.ascontiguousarray(np.concatenate(
        [C2[inv_perm].T, nsfull[inv_perm].reshape(1, NPAD)],
        axis=0).astype(ml_dtypes.float8_e4m3))  # [128, NPAD]

    # per-core, per-block, per-group chunk counts for the L3 gather
    counts = np.zeros((N_CORES, BLOCKS, 2), np.int64)
    per_core = []
    for c in range(N_CORES):
        base = c * NPC
        m = (pdst >= base) & (pdst < base + NPC)
        es, ed = psrc[m], pdst[m]
        dloc = ed - base
        blk = dloc >> 7
        erow = remap(es)  # es already positions
        half = (erow >= G0E).astype(np.int64)
        order = np.lexsort((erow, half, blk))
        erow, dloc, blk, half = (
            erow[order], dloc[order], blk[order], half[order])
        for b in range(BLOCKS):
            mb = blk == b
            counts[c, b, 0] = np.count_nonzero(mb & (half == 0))
            counts[c, b, 1] = np.count_nonzero(mb & (half == 1))
        per_core.append((erow, dloc, blk, half))

    K0 = np.maximum(1, np.ceil(counts[:, :, 0] / P).max(axis=0).astype(np.int64))
    K1 = np.ceil(counts[:, :, 1] / P).max(axis=0).astype(np.int64)
    KA = int(K0.sum())
    KB = int(K1.sum())

    cnt = np.bincount(graph_ids, minlength=N_GRAPHS).astype(np.float64)

    core_arrays = []
    for c in range(N_CORES):
        erow, dloc, blk, half = per_core[c]
        base = c * NPC
        idxA = np.zeros(KA * P, np.int32)
        dvA = np.full(KA * P, -1.0, np.float32)
        idxB = np.zeros(KB * P, np.int32)
        dvB = np.full(KB * P, -1.0, np.float32)
        offA = 0
        offB = 0
        for b in range(BLOCKS):
            for h, (idxs, dvs, K, off) in enumerate((
                (idxA, dvA, int(K0[b]), offA),
                (idxB, dvB, int(K1[b]), offB),
            )):
                m = (blk == b) & (half == h)
                n = int(np.count_nonzero(m))
                assert n <= K * P
                sl = slice(off, off + n)
                idxs[sl] = erow[m] - (0 if h == 0 else G0E)
                dvs[sl] = (dloc[m] - b * P).astype(np.float32)
                # padding stays idx=0, dstv=-1 (S row all-zero)
            offA += int(K0[b]) * P
            offB += int(K1[b]) * P

        assert G0E <= 32768 and NPAD - G0E <= 32768  # i16 gather idx

        def idx_layout(v):
            # index i -> partition i%16, column i//16, replicated on host to
            # 128 partitions so the device load is a single plain DMA
            t = v.astype(np.int16).reshape(-1, 16).T  # [16, L/16]
            return np.ascontiguousarray(np.tile(t, (8, 1)))  # [128, L/16]

        def col_layout(v, dt=np.float32):
            return np.ascontiguousarray(v.reshape(-1, P).T.astype(dt))

        own = inv_pos[np.arange(base, base + NPC)]
        real = own < N_NODES
        gph = np.full(NPC, -1.0, np.float32)
        gph[real] = graph_ids[own[real]].astype(np.float32)

        actsc = np.zeros(NPC, np.float64)
        actsc[real] = nd[own[real]] / np.maximum(
            cnt[graph_ids[own[real]]], 1.0)
        invnd = np.zeros(NPC, np.float64)
        invnd[real] = 1.0 / nd[own[real]]

        core_arrays.append(dict(
            idxA=idx_layout(idxA), idxB=idx_layout(idxB),
            dvA=col_layout(dvA, np.float16), dvB=col_layout(dvB, np.float16),
            gphv=np.ascontiguousarray(gph.reshape(BLOCKS, P).T),
            actsc=np.ascontiguousarray(
                actsc.reshape(BLOCKS, P).T.astype(np.float32)),
            invnd=np.ascontiguousarray(
                invnd.astype(np.float16).reshape(1, NPC)),
        ))

    # qm2 [2, B]: row0 = qb grid, row1 = 1.0 -- replicated input
    qm2 = np.zeros((2, B), np.float64)
    qm2[0] = qb
    qm2[1] = 1.0
    qm2 = qm2.astype(np.float16)

    sched = dict(K0=K0, K1=K1, KA=KA, KB=KB)
    common = dict(c2t=c2t, qm2=qm2)
    return sched, core_arrays, common


def build_nc(sched, reps=1, with_coll=True, with_gather=True,
             with_sgen=True, with_compute=True, msg_bufs=34, sgen_bufs=26,
             hbuf_bufs=6, ct_bufs=4):
    """Build and compile the 8-core SPMD Bass program."""
    K0, K1, KA, KB = sched["K0"], sched["K1"], sched["KA"], sched["KB"]
    KBx = max(KB, 1)
    NGT = N_GRAPHS // P  # 4

    nc = bacc.Bacc("TRN2", target_bir_lowering=False, debug=False,
                   num_devices=N_CORES, num_swdge_queues=4)

    def inp(name, shape, dt=F32):
        return nc.dram_tensor(name, list(shape), dt, kind="ExternalInput").ap()

    d_idxA = inp("idxA", [P, KA * 8], I16)
    d_idxB = inp("idxB", [P, KBx * 8], I16)
    d_dvA = inp("dvA", [P, KA], F16)
    d_dvB = inp("dvB", [P, KBx], F16)
    d_qm2 = inp("qm2", [2, B], F16)
    d_gph = inp("gphv", [P, BLOCKS])
    d_actsc = inp("actsc", [P, BLOCKS])
    d_invnd = inp("invnd", [1, NPC], F16)
    d_c2t = inp("c2t", [B + 1, NPAD], F8)
    d_W0b0 = inp("W0b0", [2, HID], F16)

    d_W1 = inp("W1b", [HID, HID], F16)
    d_W2 = inp("W2b", [HID, HID], F16)
    d_Wc = inp("Wc", [HID, N_CLASSES])
    d_b1row = inp("b1row", [1, HID], F16)
    d_b2row = inp("b2row", [1, HID], F16)
    d_bcr8 = inp("bcr8", [1, N_CLASSES])

    out = nc.dram_tensor("out", [N_GRAPHS, N_CLASSES], F32,
                         kind="ExternalOutput").ap()

    table2a = nc.dram_tensor("table2a", [G0E, HID], TDT, kind="Internal").ap()
    table2b = nc.dram_tensor("table2b", [NPAD - G0E, HID], TDT,
                             kind="Internal").ap()
    partial = nc.dram_tensor("partial", [N_GRAPHS, N_CLASSES], F32,
                             kind="Internal").ap()
    gathf = nc.dram_tensor("gathf", [N_CORES * N_GRAPHS, N_CLASSES], F32,
                           kind="Internal", addr_space="Shared").ap()

    RG = [list(range(N_CORES))]

    # block -> chunk ranges in streams A and B
    offA = np.concatenate([[0], np.cumsum(K0)]).astype(int)
    offB = np.concatenate([[0], np.cumsum(K1)]).astype(int)

    with tile.TileContext(nc) as tc:
        with tc.tile_pool(name="const", bufs=1) as cp, \
             tc.tile_pool(name="msg", bufs=msg_bufs) as mp, \
             tc.tile_pool(name="sgen", bufs=sgen_bufs) as sp, \
             tc.tile_pool(name="hbuf", bufs=hbuf_bufs) as hp, \
             tc.tile_pool(name="ctp", bufs=ct_bufs) as ctp, \
             tc.tile_pool(name="agg_ps", bufs=3, space="PSUM") as agg_ps, \
             tc.tile_pool(name="p2_ps", bufs=2, space="PSUM") as p2p, \
             tc.tile_pool(name="z_ps", bufs=2, space="PSUM") as zp, \
             tc.tile_pool(name="r_ps", bufs=1, space="PSUM") as r_ps:

            def load_const(ap_in, shape, dt=F32):
                t = cp.tile(list(shape), dt, tag=ap_in.name)
                nc.sync.dma_start(t[:], ap_in[:])
                return t

            idxA = load_const(d_idxA, [P, KA * 8], I16)
            idxB = load_const(d_idxB, [P, KBx * 8], I16)
            dvA = load_const(d_dvA, [P, KA], F16)
            dvB = load_const(d_dvB, [P, KBx], F16)
            gph = load_const(d_gph, [P, BLOCKS])
            actsc = load_const(d_actsc, [P, BLOCKS])
            invnd = load_const(d_invnd, [1, NPC], F16)
            qm2 = load_const(d_qm2, [2, B], F16)
            W0b0 = load_const(d_W0b0, [2, HID], F16)
            W1b = load_const(d_W1, [HID, HID], F16)
            W2b = load_const(d_W2, [HID, HID], F16)
            Wc = load_const(d_Wc, [HID, N_CLASSES])
            b1row = load_const(d_b1row, [1, HID], F16)
            b2row = load_const(d_b2row, [1, HID], F16)
            bcr8 = load_const(d_bcr8, [1, N_CLASSES])
            ones1 = cp.tile([1, P], F32, tag="ones1")
            nc.vector.memset(ones1[:], 1.0)

            iota_i = cp.tile([P, P], I32, tag="iota_i")
            nc.gpsimd.iota(iota_i[:], pattern=[[1, P]], base=0,
                           channel_multiplier=0)
            iota_b = cp.tile([P, P], F16, tag="iota_b")
            nc.vector.tensor_copy(iota_b[:], iota_i[:])
            iotg_i = cp.tile([P, N_GRAPHS], I32, tag="iotg_i")
            nc.gpsimd.iota(iotg_i[:], pattern=[[1, N_GRAPHS]], base=0,
                           channel_multiplier=0)
            iotg_f = cp.tile([P, N_GRAPHS], F16, tag="iotg_f")
            nc.vector.tensor_copy(iotg_f[:], iotg_i[:])

            RELU = mybir.ActivationFunctionType.Relu
            COPY = mybir.ActivationFunctionType.Copy

            # block -> chunk list over both streams
            def block_chunks(b):
                res = []
                for ca in range(offA[b], offA[b + 1]):
                    res.append(("A", ca))
                for cb in range(offB[b], offB[b + 1]):
                    res.append(("B", cb))
                return res

            # Global gather-instruction counter: msg pool slot = count %
            # msg_bufs, SWDGE queue = count % 4 stays consistent per slot.
            gather_count = [0]

            def emit_gathers():
                """Gather msgs per group, block-sorted across streams."""
                chunk_src = {}
                groups = []
                blockA = np.searchsorted(offA[1:], np.arange(KA),
                                         side="right")
                blockB = np.searchsorted(offB[1:], np.arange(KBx),
                                         side="right")
                for stream, K, idx_t, blk_of in (
                        ("A", KA, idxA, blockA), ("B", KB, idxB, blockB)):
                    base_ap = table2a[:] if stream == "A" else table2b[:]
                    g0 = 0
                    while g0 < K:
                        ln = min(GA, K - g0)
                        groups.append(
                            (int(blk_of[g0]), stream, g0, ln, base_ap,
                             idx_t))
                        g0 += ln
                groups.sort(key=lambda g: (g[0], g[1]))
                for _fb, stream, g0, ln, base_ap, idx_t in groups:
                    gi = gather_count[0]
                    gather_count[0] += 1
                    mt = mp.tile([P, GA * P], TDT, tag="msg")
                    out_ap = mt[:][:, :ln * P].rearrange(
                        "p (a b) -> p a b", b=P)
                    if with_gather:
                        nc.gpsimd.dma_gather(
                            out_ap=out_ap, in_ap=base_ap,
                            idxs_ap=idx_t[:][:, g0 * 8:(g0 + ln) * 8],
                            num_idxs=ln * P, num_idxs_reg=ln * P,
                            elem_size=HID, queue_num=gi % 4)
                    else:  # ablation stub: tiny write so the tile is owned
                        eng = (nc.sync, nc.scalar)[gi % 2]
                        eng.dma_start(mt[:][:, 0:8], d_dvA[:, 0:8])
                    dv = dvA if stream == "A" else dvB
                    S8 = sp.tile([P, GA * P], F8, tag="S8")
                    s_ap = S8[:][:, :ln * P].rearrange(
                        "p (a b) -> p a b", b=P)
                    if with_sgen:
                        io8 = iota_b[:].unsqueeze(1).broadcast_to(
                            [P, ln, P])
                        dv8 = dv[:][:, g0:g0 + ln].unsqueeze(2). \
                            broadcast_to([P, ln, P])
                        nc.vector.tensor_tensor(
                            out=s_ap, in0=io8, in1=dv8,
                            op=mybir.AluOpType.is_equal)
                    else:  # ablation stub
                        eng = (nc.scalar, nc.sync)[gi % 2]
                        eng.dma_start(S8[:][:, 0:8], d_dvA[:, 0:8])
                    for j in range(ln):
                        chunk_src[(stream, g0 + j)] = (mt, S8, j)
                return chunk_src

            for rep in range(reps):
                # -------- prologue: TT = relu(W0b0^T qm2); TW1 --------
                # TW1 rows 0..126 = (relu table)^T W1; row 127 = b1 (pairs
                # with c2t row 127 = ns for the fused bias outer product).
                TW1_sb = cp.tile([B + 1, HID], F16, tag="tw1")
                if with_compute:
                    TTps = zp.tile([P, 4 * P], F32, tag="zps")
                    nc.tensor.matmul(out=TTps[:][:, :B], lhsT=W0b0[:],
                                     rhs=qm2[:], start=True, stop=True)
                    TT_sb = cp.tile([P, B], F16, tag="hT1")
                    nc.scalar.activation(out=TT_sb[:], in_=TTps[:][:, :B],
                                         func=RELU, bias=0.0, scale=1.0)
                    TW1ps = zp.tile([P, 4 * P], F32, tag="zps")
                    nc.tensor.matmul(out=TW1ps[:][0:B, :HID], lhsT=TT_sb[:],
                                     rhs=W1b[:], start=True, stop=True)
                    # rows 0..B-1 = TW1; row B (base 32, aligned) = b1
                    nc.vector.tensor_copy(TW1_sb[:][0:B, :],
                                          TW1ps[:][0:B, :HID])
                    nc.vector.tensor_copy(TW1_sb[:][B:B + 1, :], b1row[:])

                # -------- L2: full table, 16-block ct chunks --------
                zg = [0]
                for c0 in range(0, NBLK if with_compute else 0, 16):
                    cn = min(16, NBLK - c0)
                    ct = ctp.tile([B + 1, 16 * P], F8, tag="ct")
                    nc.sync.dma_start(
                        ct[:][:, :cn * P], d_c2t[:, c0 * P:(c0 + cn) * P])
                    p2_sb = hp.tile([P, 16 * P], TDT, tag="pout")
                    for z0 in range(0, cn, 4):
                        zn = min(4, cn - z0)
                        z_ps = zp.tile([P, 4 * P], F32, tag="zps")
                        nc.tensor.matmul(
                            out=z_ps[:][:, :zn * P], lhsT=TW1_sb[:],
                            rhs=ct[:][:, z0 * P:(z0 + zn) * P],
                            start=True, stop=True)
                        h2T = hp.tile([P, 4 * P], F16, tag="hT")
                        # alternate relu/copy between Act and DVE
                        # (GPSIMD cannot access PSUM)
                        if zg[0] % 2 == 0:
                            nc.scalar.activation(
                                out=h2T[:][:, :zn * P],
                                in_=z_ps[:][:, :zn * P],
                                func=RELU, bias=0.0, scale=1.0)
                        else:
                            nc.vector.tensor_scalar(
                                out=h2T[:][:, :zn * P],
                                in0=z_ps[:][:, :zn * P],
                                scalar1=0.0, scalar2=None,
                                op0=mybir.AluOpType.max)
                        p2_ps = p2p.tile([P, 4 * P], F32, tag="p2ps")
                        for j in range(zn):
                            nc.tensor.matmul(
                                out=p2_ps[:][:, j * P:(j + 1) * P],
                                lhsT=h2T[:][:, j * P:(j + 1) * P],
                                rhs=W2b[:], start=True, stop=True)
                        if zg[0] % 2 == 1:
                            nc.scalar.activation(
                                out=p2_sb[:][:, z0 * P:(z0 + zn) * P],
                                in_=p2_ps[:][:, :zn * P],
                                func=COPY, bias=0.0, scale=1.0)
                        else:
                            nc.vector.tensor_copy(
                                p2_sb[:][:, z0 * P:(z0 + zn) * P],
                                p2_ps[:][:, :zn * P])
                        zg[0] += 1
                    # one contiguous write: partition p's cn rows are
                    # consecutive in dram (write-order interleave)
                    rows0 = c0 * P
                    dst_ap = (
                        table2a[rows0:rows0 + cn * P, :]
                        if rows0 < G0E
                        else table2b[rows0 - G0E:rows0 - G0E + cn * P, :])
                    nc.sync.dma_start(
                        dst_ap.rearrange("(n w) f -> n (w f)", n=P),
                        p2_sb[:][:, :cn * P])

                # -------- L3 + readout --------
                chunk_src = emit_gathers()
                rT = r_ps.tile([P, N_GRAPHS], F32, tag="rT",
                               name=f"rT_{rep}")
                for b in range(BLOCKS if with_compute else 0):
                    chunks = block_chunks(b)
                    agg = agg_ps.tile([P, P], F32, tag="aggps")
                    for j, (stream, ci) in enumerate(chunks):
                        mt, S8, col = chunk_src[(stream, ci)]
                        nc.tensor.matmul(
                            out=agg[:],
                            lhsT=S8[:][:, col * P:(col + 1) * P],
                            rhs=mt[:][:, col * P:(col + 1) * P],
                            start=(j == 0), stop=False)
                    # bias: agg += invnd^T @ b2row (z3 = nd*agg later)
                    nc.tensor.matmul(
                        out=agg[:], lhsT=invnd[:][:, b * P:(b + 1) * P],
                        rhs=b2row[:], start=False, stop=True)
                    # h3 = relu(agg * actsc), actsc = nd * invc > 0
                    h3 = hp.tile([P, P], F16, tag="h3")
                    nc.scalar.activation(out=h3[:], in_=agg[:],
                                         func=RELU, bias=0.0,
                                         scale=actsc[:][:, b:b + 1])
                    # readout: Sg [dst, 512] one-hot; rT += h3^T @ Sg
                    Sg = sp.tile([P, N_GRAPHS], F16, tag="Sg4")
                    nc.vector.tensor_scalar(
                        out=Sg[:], in0=iotg_f[:],
                        scalar1=gph[:][:, b:b + 1], scalar2=None,
                        op0=mybir.AluOpType.is_equal,
                        op1=mybir.AluOpType.bypass)
                    nc.tensor.matmul(
                        out=rT[:], lhsT=h3[:], rhs=Sg[:],
                        start=(b == 0), stop=(b == BLOCKS - 1))

                # -------- head (pre-AR): part = rT^T Wc + bc/8 --------
                if with_compute:
                    r_sb = cp.tile([P, N_GRAPHS], F32, tag="rsb")
                    nc.vector.tensor_copy(r_sb[:], rT[:])
                    for t in range(NGT):
                        o_ps = zp.tile([P, 4 * P], F32, tag="zps")
                        nc.tensor.matmul(
                            out=o_ps[:][:, :N_CLASSES],
                            lhsT=r_sb[:][:, t * P:(t + 1) * P],
                            rhs=Wc[:], start=True, stop=False)
                        nc.tensor.matmul(
                            out=o_ps[:][:, :N_CLASSES], lhsT=ones1[:],
                            rhs=bcr8[:], start=False, stop=True)
                        o_sb = cp.tile([P, N_CLASSES], F32, tag=f"osb_{t}")
                        nc.vector.tensor_copy(o_sb[:],
                                              o_ps[:][:, :N_CLASSES])
                        nc.sync.dma_start(
                            partial[t * P:(t + 1) * P, :], o_sb[:])

                if with_coll and with_compute:
                    nc.gpsimd.collective_compute(
                        "AllGather", mybir.AluOpType.bypass,
                        replica_groups=RG,
                        ins=[partial[:]], outs=[gathf[:]])
                    gview = gathf.rearrange("(c q) x -> q c x", c=N_CORES)
                    for t in range(NGT):
                        g8 = cp.tile([P, N_CORES, N_CLASSES], F32,
                                     tag=f"g8_{t}")
                        nc.sync.dma_start(
                            g8[:], gview[t * P:(t + 1) * P])
                        acc = cp.tile([P, N_CLASSES], F32, tag=f"acc_{t}")
                        nc.vector.tensor_tensor(
                            out=acc[:], in0=g8[:][:, 0, :],
                            in1=g8[:][:, 1, :], op=mybir.AluOpType.add)
                        for c in range(2, N_CORES):
                            nc.vector.tensor_tensor(
                                out=acc[:], in0=acc[:],
                                in1=g8[:][:, c, :],
                                op=mybir.AluOpType.add)
                        nc.sync.dma_start(out[t * P:(t + 1) * P, :],
                                          acc[:])

    nc.compile()
    return nc


def make_in_maps(core_arrays, common, W0, b0, W1, b1, W2, b2, Wc, bc):
    W0 = np.asarray(W0, np.float32).reshape(1, HID)
    b0 = np.asarray(b0, np.float32).reshape(1, HID)
    cm = dict(
        qm2=common["qm2"], c2t=common["c2t"],
        W0b0=np.ascontiguousarray(
            np.concatenate([W0, b0], axis=0).astype(np.float16)),
        W1b=np.ascontiguousarray(np.asarray(W1, np.float32).astype(np.float16)),
        W2b=np.ascontiguousarray(np.asarray(W2, np.float32).astype(np.float16)),
        Wc=np.ascontiguousarray(Wc, np.float32),
        b1row=np.ascontiguousarray(
            np.asarray(b1, np.float32).reshape(1, HID).astype(np.float16)),

        b2row=np.ascontiguousarray(
            np.asarray(b2, np.float32).reshape(1, HID).astype(np.float16)),
        bcr8=np.ascontiguousarray(
            np.asarray(bc, np.float32).reshape(1, N_CLASSES) / N_CORES),
    )
    in_maps = []
    for c in range(N_CORES):
        m = dict(cm)
        ca = core_arrays[c]
        for k in ("idxA", "idxB", "dvA", "dvB", "gphv", "actsc", "invnd"):
            m[k] = ca[k]
        in_maps.append(m)
    return in_maps


_CACHE = {}


def _get_compiled(src, dst, graph_ids):
    import hashlib
    h = hashlib.md5()
    h.update(np.asarray(src).tobytes())
    h.update(np.asarray(dst).tobytes())
    h.update(np.asarray(graph_ids).tobytes())
    key = h.hexdigest()
    if key not in _CACHE:
        sched, core_arrays, common = _prep_graph(src, dst, graph_ids)
        nc = build_nc(sched)
        _CACHE[key] = (nc, core_arrays, common)
    return _CACHE[key]


def kernel(W0, b0, W1, b1, W2, b2, Wc, bc, src, dst, graph_ids,
           num_graphs=None, **_ignored):
    nc, core_arrays, common = _get_compiled(src, dst, graph_ids)
    in_maps = make_in_maps(core_arrays, common, W0, b0, W1, b1, W2, b2,
                           Wc, bc)
    res = bass_utils.run_bass_kernel_spmd(
        nc, in_maps, core_ids=list(range(N_CORES)))
    o = res.results[0]["out"]
    return np.asarray(o, np.float32)
